# revision 29
# baseline (speedup 1.0000x reference)
"""Trainium2 Bass kernel for nn_Attention_21285857919576.

Strategy: 8 cores = 4 batches x 2 head-groups (tensor parallel over heads).
Each core computes, for its (batch b, head-group g):
  - Q/K/V projections (s-major) on TensorE in bf16
  - per-head RMSNorm (rsqrt via ACT exp(-0.5*ln(ms))) + RoPE on DVE/ACT/GpSimd
  - DMA-transpose roped Q,K into d-major slabs (QT/KT)
  - causal attention in transposed orientation:
      scoresT[k,q] = KT_tile.T @ QT  -> exp (no max subtraction: RMS+RoPE
      bounds |scores| <= sqrt(D)) -> p (bf16)
      outT[d,q]  += V_tile.T @ p     (V is s-major from the projection)
      den[1,q]   += ones.T @ p
      attnoutT = outT * (1/den broadcast via K=1 matmul)
  - partial output = attnoutT.T @ woT (this group's wo columns)
Host sums the two partial outputs per batch. No collectives; the 8 cores are
fully independent and perfectly load balanced.

All large matmuls run in bf16 (fp32 PSUM accumulate). Host-side
preprocessing folds q/k norm weights, the rotate-half sign, and the
1/sqrt(D) score scale into the cos/sin tables, and pre-transposes
x and the weights so every DMA is layout-natural.
"""
import sys
import numpy as np
import ml_dtypes

for _p in ("/opt/trn_rl_repo", "/opt/pypackages"):
    if _p not in sys.path:
        sys.path.append(_p)

import concourse.bass as bass
from concourse import bacc, mybir, tile
from concourse.bass_utils import run_bass_kernel_spmd


def _install_ntff_hook_shim():
    """The staged antenv package lacks axon_hooks; provide it so
    run_bass_kernel_spmd(trace=True) can drive NTFF profiling via the
    injected libaxon .so (same mechanism trn_boot would register)."""
    import types
    if "antenv.axon_hooks" in sys.modules:
        return
    mod = types.ModuleType("antenv.axon_hooks")
    _state = {"hook": None}
    mod.set_axon_ntff_profile_hook = lambda h: _state.__setitem__("hook", h)
    mod.get_axon_ntff_profile_hook = lambda: _state["hook"]
    sys.modules["antenv.axon_hooks"] = mod
    try:
        import antenv
        antenv.axon_hooks = mod
    except ImportError:
        pass
    try:
        from trn_agent_boot.trn_boot import _ntff_profile_via_ctypes
        import os
        so = "/opt/axon/libaxon_pjrt.so"
        if os.path.exists(so):
            mod.set_axon_ntff_profile_hook(_ntff_profile_via_ctypes(so))
    except Exception:
        pass


_install_ntff_hook_shim()


def _install_act_table_patch():
    """Force Exp/Ln/Copy/Square onto the single natural_log_exp_and_others
    ACT table set: blank every other set containing exp/ln so the
    table-load chooser can't alternate between sets (each switch costs
    ~2.7us and we interleave Ln (rms) with Exp (softmax))."""
    import concourse.hw_specs as hw_specs
    import concourse.bacc as bacc_mod
    if getattr(hw_specs, "_act_patch", False):
        return
    orig = hw_specs.get_activation_tables

    def patched(module_arch):
        tables = orig(module_arch)
        keep = "natural_log_exp_and_others"
        if keep in tables:
            for name, fns in tables.items():
                if name != keep and any(f.name in ("Exp", "Ln") for f in fns):
                    tables[name] = set()
        return tables

    hw_specs.get_activation_tables = patched
    if getattr(bacc_mod, "get_activation_tables", None) is orig:
        bacc_mod.get_activation_tables = patched
    hw_specs._act_patch = True


_install_act_table_patch()

BF = ml_dtypes.bfloat16
F32 = mybir.dt.float32
BF16 = mybir.dt.bfloat16
ALU = mybir.AluOpType
AF = mybir.ActivationFunctionType

S, HSD, D = 2048, 2048, 128
NQ, NKV = 8, 4          # per-core q heads / kv heads
NT = S // 128           # 16 s-tiles
EPS = 1e-6
MASKVAL = -1.0e9


def _emit(tc, aps):
    nc = tc.nc
    xT = aps["xT"]
    wqT = aps["wqT"]
    wkT = aps["wkT"]
    wvT = aps["wvT"]
    woT = aps["woT"]
    cosq = aps["cosq"]
    sinq = aps["sinq"]
    cosk = aps["cosk"]
    sink = aps["sink"]
    maskd = aps["mask"]
    outd = aps["out"]

    xT3 = xT.rearrange("(ho hi) s -> hi ho s", hi=128)       # [128, 16, 2048]
    wqT3 = wqT.rearrange("(ho hi) f -> hi ho f", hi=128)     # [128, 16, 1024]
    wkT3 = wkT.rearrange("(ho hi) f -> hi ho f", hi=128)     # [128, 16, 512]
    wvT3 = wvT.rearrange("(ho hi) f -> hi ho f", hi=128)
    woT3 = woT.rearrange("(fo fi) o -> fi fo o", fi=128)     # [128, 8, 2048]

    from contextlib import ExitStack
    with ExitStack() as ctx:
        singles = ctx.enter_context(tc.tile_pool(name="singles", bufs=1))
        wsl = ctx.enter_context(tc.tile_pool(name="wsl", bufs=2))
        xtp = ctx.enter_context(tc.tile_pool(name="xtp", bufs=2))
        trig = ctx.enter_context(tc.tile_pool(name="trig", bufs=4))
        qfp = ctx.enter_context(tc.tile_pool(name="qfp", bufs=3))
        t12 = ctx.enter_context(tc.tile_pool(name="t12", bufs=4))
        sqp = ctx.enter_context(tc.tile_pool(name="sqp", bufs=2))
        tiny = ctx.enter_context(tc.tile_pool(name="tiny", bufs=6))
        qsbp = ctx.enter_context(tc.tile_pool(name="qsbp", bufs=2))
        ksbp = ctx.enter_context(tc.tile_pool(name="ksbp", bufs=2))
        pp = ctx.enter_context(tc.tile_pool(name="pp", bufs=6))
        denp = ctx.enter_context(tc.tile_pool(name="denp", bufs=2))
        outp = ctx.enter_context(tc.tile_pool(name="outp", bufs=3))
        rdenp = ctx.enter_context(tc.tile_pool(name="rdenp", bufs=2))
        diagp = ctx.enter_context(tc.tile_pool(name="diagp", bufs=4))
        psA = ctx.enter_context(tc.tile_pool(name="psA", bufs=2, space="PSUM"))
        psSC = ctx.enter_context(tc.tile_pool(name="psSC", bufs=2, space="PSUM"))
        psO = ctx.enter_context(tc.tile_pool(name="psO", bufs=2, space="PSUM"))
        psD = ctx.enter_context(tc.tile_pool(name="psD", bufs=2, space="PSUM"))

        QT = singles.tile([128, NQ, S], BF16)    # [d, h, s]
        KT = singles.tile([128, NKV, S], BF16)   # [d, kv, s]
        Vs = singles.tile([128, NT, NKV * D], BF16)  # [s_i, s_o, f]
        AOT = singles.tile([128, NQ, S], BF16)   # [d, h, s]
        mask_sb = singles.tile([128, 128], F32)
        ones_col = singles.tile([128, 1], BF16)
        ones_row = singles.tile([1, 128], F32)

        eps_sb = singles.tile([128, 1], F32)
        ident = singles.tile([128, 128], F32)
        nc.sync.dma_start(out=mask_sb[:], in_=maskd)
        nc.vector.memset(ones_col[:], 1.0)
        nc.vector.memset(ones_row[:], 1.0)
        nc.vector.memset(eps_sb[:], EPS)
        from concourse.masks import make_identity
        make_identity(nc, ident[:])

        def process_qk(ps, cos_t, sin_t, out_ap):
            """RoPE 4 heads from psum tile ps [128,4,128] (unnormalized),
            write bf16 to out_ap [128, 512]; return rms_inv [128, 4] tile
            (applied later via the diag transpose matmul)."""
            qf = qfp.tile([128, 4, 128], F32, tag="qf")
            nc.scalar.copy(qf[:], ps[:])          # ACT: psum -> sbuf fp32
            sq = sqp.tile([128, 4, 128], F32, tag="sq")
            ss = tiny.tile([128, 4], F32, tag="ss")
            for hb in range(4):
                nc.vector.scalar_tensor_tensor(
                    out=sq[:, hb], in0=qf[:, hb], scalar=1.0, in1=qf[:, hb],
                    op0=ALU.mult, op1=ALU.mult, accum_out=ss[:, hb:hb + 1])
            lnt = tiny.tile([128, 4], F32, tag="lnt")
            nc.scalar.activation(lnt[:], ss[:], AF.Ln, scale=1.0 / D,
                                 bias=eps_sb[:])
            rmsi = tiny.tile([128, 4], F32, tag="rmsi")
            nc.scalar.activation(rmsi[:], lnt[:], AF.Exp, scale=-0.5)
            cosb = cos_t[:].unsqueeze(1).to_broadcast([128, 4, 128])
            sinb = sin_t[:].unsqueeze(1).to_broadcast([128, 4, 128])
            t1 = t12.tile([128, 4, 128], F32, tag="t1")
            t2 = t12.tile([128, 4, 128], F32, tag="t2")
            nc.vector.tensor_tensor(out=t1[:], in0=qf[:], in1=cosb, op=ALU.mult)
            qfb = qf[:]
            rot = bass.AP(tensor=qfb.tensor, offset=qfb.offset + 64,
                          ap=[qfb.ap[0], [128, 4], [-64, 2], [1, 64]])
            t2v = t2[:].rearrange("p h (j d) -> p h j d", j=2)
            sinv = sinb.rearrange("p h (j d) -> p h j d", j=2)
            nc.vector.tensor_tensor(out=t2v, in0=rot, in1=sinv, op=ALU.mult)
            nc.gpsimd.tensor_tensor(out=out_ap, in0=t1[:], in1=t2[:], op=ALU.add)
            return rmsi

        def transpose4(src_ap, rmsi, dst_ap):
            """Transpose+normalize 4 head-tiles: src_ap [128 s, 512 f] bf16
            (roped, unnormalized), rmsi [128 s, 4], into dst_ap
            [128 d, 4, 128 s] (bf16 slab slice) via PE diag matmuls."""
            pst = psSC.tile([128, 4, 128], F32, tag="sc")
            diag4 = diagp.tile([128, 4, 128], BF16, tag="diag")
            identb = ident[:].unsqueeze(1).to_broadcast([128, 4, 128])
            rmsib = rmsi[:].unsqueeze(2).to_broadcast([128, 4, 128])
            nc.vector.tensor_tensor(out=diag4[:], in0=identb, in1=rmsib,
                                    op=ALU.mult)
            for hb in range(4):
                nc.tensor.matmul(pst[:, hb],
                                 lhsT=src_ap[:, hb * 128:(hb + 1) * 128],
                                 rhs=diag4[:, hb], start=True, stop=True)
            nc.scalar.copy(dst_ap, pst[:])

        # ---- sweep 1: Q projection + rms/rope + transpose into QT ----
        wqA = wsl.tile([128, 16, 512], BF16, tag="w")
        wqB = wsl.tile([128, 16, 512], BF16, tag="w")
        nc.sync.dma_start(out=wqA[:], in_=wqT3[:, :, 0:512])
        nc.sync.dma_start(out=wqB[:], in_=wqT3[:, :, 512:1024])
        for st in range(NT):
            sl = slice(st * 128, (st + 1) * 128)
            xt = xtp.tile([128, 16, 128], BF16, tag="xt")
            nc.sync.dma_start(out=xt[:], in_=xT3[:, :, sl])
            ps0 = psA.tile([128, 4, 128], F32, tag="psA")
            ps1 = psA.tile([128, 4, 128], F32, tag="psA")
            for hs in range(16):
                nc.tensor.matmul(ps0[:], lhsT=xt[:, hs], rhs=wqA[:, hs],
                                 start=(hs == 0), stop=(hs == 15))
                nc.tensor.matmul(ps1[:], lhsT=xt[:, hs], rhs=wqB[:, hs],
                                 start=(hs == 0), stop=(hs == 15))
            ct = trig.tile([128, 128], F32, tag="trig")
            stt = trig.tile([128, 128], F32, tag="trig")
            nc.sync.dma_start(out=ct[:], in_=cosq[sl, :])
            nc.sync.dma_start(out=stt[:], in_=sinq[sl, :])
            qsb = qsbp.tile([128, 1024], BF16, tag="qsb")
            rmsi0 = process_qk(ps0, ct, stt, qsb[:, 0:512])
            rmsi1 = process_qk(ps1, ct, stt, qsb[:, 512:1024])
            transpose4(qsb[:, 0:512], rmsi0, QT[:, 0:4, sl])
            transpose4(qsb[:, 512:1024], rmsi1, QT[:, 4:8, sl])

        # ---- sweep 2: K/V projections; K rms/rope + transpose; V copy ----
        wkS = wsl.tile([128, 16, 512], BF16, tag="w")
        wvS = wsl.tile([128, 16, 512], BF16, tag="w")
        nc.sync.dma_start(out=wkS[:], in_=wkT3[:])
        nc.sync.dma_start(out=wvS[:], in_=wvT3[:])
        for st in range(NT):
            sl = slice(st * 128, (st + 1) * 128)
            xt = xtp.tile([128, 16, 128], BF16, tag="xt")
            nc.sync.dma_start(out=xt[:], in_=xT3[:, :, sl])
            psk = psA.tile([128, 4, 128], F32, tag="psA")
            psv = psA.tile([128, 4, 128], F32, tag="psA")
            for hs in range(16):
                nc.tensor.matmul(psk[:], lhsT=xt[:, hs], rhs=wkS[:, hs],
                                 start=(hs == 0), stop=(hs == 15))
                nc.tensor.matmul(psv[:], lhsT=xt[:, hs], rhs=wvS[:, hs],
                                 start=(hs == 0), stop=(hs == 15))
            ct = trig.tile([128, 128], F32, tag="trig")
            stt = trig.tile([128, 128], F32, tag="trig")
            nc.sync.dma_start(out=ct[:], in_=cosk[sl, :])
            nc.sync.dma_start(out=stt[:], in_=sink[sl, :])
            ksb = ksbp.tile([128, 512], BF16, tag="ksb")
            rmsik = process_qk(psk, ct, stt, ksb[:, 0:512])
            nc.scalar.copy(Vs[:, st, :], psv[:])
            transpose4(ksb[:, 0:512], rmsik, KT[:, 0:4, sl])

        # ---- attention (transposed orientation) + wo, interleaved by qc ----
        for qc in range(4):
            qbase = qc * 512
            for h in range(NQ):
                kv = h // 2
                nkt = 4 * qc + 4
                pso = psO.tile([128, 512], F32, tag="psO")
                psden = psD.tile([1, 512], F32, tag="psD")
                for kt in range(nkt):
                    j = kt - 4 * qc
                    lo = 128 * j if j >= 0 else 0
                    psc = psSC.tile([128, 512], F32, tag="sc")
                    nc.tensor.matmul(
                        psc[:, lo:512],
                        lhsT=KT[:, kv, kt * 128:(kt + 1) * 128],
                        rhs=QT[:, h, qbase + lo:qbase + 512],
                        start=True, stop=True)
                    if j >= 0:
                        nc.vector.tensor_tensor(
                            out=psc[:, lo:lo + 128], in0=psc[:, lo:lo + 128],
                            in1=mask_sb[:], op=ALU.add)
                    p = pp.tile([128, 512], BF16, tag="p")
                    nc.scalar.activation(p[:, lo:512], psc[:, lo:512], AF.Exp)
                    nc.tensor.matmul(
                        pso[:, lo:512],
                        lhsT=Vs[:, kt, kv * 128:(kv + 1) * 128],
                        rhs=p[:, lo:512],
                        start=(kt == 0), stop=(kt == nkt - 1))
                    nc.tensor.matmul(
                        psden[:, lo:512], lhsT=ones_col[:], rhs=p[:, lo:512],
                        start=(kt == 0), stop=(kt == nkt - 1))
                den_sb = denp.tile([1, 512], F32, tag="den")
                nc.vector.reciprocal_approx_fast(out=den_sb[:], in_=psden[:])
                rden = rdenp.tile([128, 512], F32, tag="rden")
                nc.gpsimd.partition_broadcast(rden[:], den_sb[:])
                nc.vector.tensor_tensor(
                    out=AOT[:, h, qbase:qbase + 512], in0=pso[:], in1=rden[:],
                    op=ALU.mult)

            # wo for the 4 s-tiles covered by this qc block
            if qc == 0:
                woA = wsl.tile([128, 4, 2048], BF16, tag="w")
                woB = wsl.tile([128, 4, 2048], BF16, tag="w")
                nc.sync.dma_start(out=woA[:], in_=woT3[:, 0:4, :])
                nc.sync.dma_start(out=woB[:], in_=woT3[:, 4:8, :])
            for st in range(4 * qc, 4 * qc + 4):
                ssl = slice(st * 128, (st + 1) * 128)
                for oc in range(4):
                    osl = slice(oc * 512, (oc + 1) * 512)
                    po = psO.tile([128, 512], F32, tag="psO")
                    for fc in range(NQ):
                        w = woA if fc < 4 else woB
                        nc.tensor.matmul(po[:], lhsT=AOT[:, fc, ssl],
                                         rhs=w[:, fc % 4, osl],
                                         start=(fc == 0), stop=(fc == 7))
                    osb = outp.tile([128, 512], F32, tag="osb")
                    nc.vector.tensor_copy(out=osb[:], in_=po[:])
                    nc.sync.dma_start(out=outd[ssl, osl], in_=osb[:])


def build_program():
    nc = bacc.Bacc("TRN2", target_bir_lowering=False, debug=False,
                   num_devices=8)
    shapes = {
        "xT": ([HSD, S], BF16), "wqT": ([HSD, NQ * D], BF16),
        "wkT": ([HSD, NKV * D], BF16), "wvT": ([HSD, NKV * D], BF16),
        "woT": ([NQ * D, HSD], BF16),
        "cosq": ([S, D], F32), "sinq": ([S, D], F32),
        "cosk": ([S, D], F32), "sink": ([S, D], F32),
        "mask": ([128, 128], F32),
    }
    aps = {n: nc.dram_tensor(n, sh, dt, kind="ExternalInput").ap()
           for n, (sh, dt) in shapes.items()}
    aps["out"] = nc.dram_tensor("out", [S, HSD], F32,
                                kind="ExternalOutput").ap()

    with tile.TileContext(nc) as tc:
        _emit(tc, aps)
    nc.compile()
    return nc


def make_in_maps(x, cos, sin, wq, wk, wv, wo, q_norm_w, k_norm_w):
    """Host-side preprocessing + sharding into 8 per-core input maps."""
    sign = np.where(np.arange(D) < 64, -1.0, 1.0).astype(np.float32)
    wrot_q = q_norm_w[(np.arange(D) + 64) % D]
    wrot_k = k_norm_w[(np.arange(D) + 64) % D]
    rsd = 1.0 / np.sqrt(np.float32(D))
    cos_q = (cos * q_norm_w[None, :] * rsd).astype(np.float32)
    sin_q = (sin * sign[None, :] * wrot_q[None, :] * rsd).astype(np.float32)
    cos_k = (cos * k_norm_w[None, :]).astype(np.float32)
    sin_k = (sin * sign[None, :] * wrot_k[None, :]).astype(np.float32)
    ii = np.arange(128)
    mask = np.where(ii[None, :] >= ii[:, None], 0.0, MASKVAL).astype(np.float32)

    def bf(a):
        return np.ascontiguousarray(a).astype(BF)

    in_maps = []
    for ci in range(8):
        b, g = ci // 2, ci % 2
        in_maps.append({
            "xT": bf(x[b].T),
            "wqT": bf(wq[g * 1024:(g + 1) * 1024, :].T),
            "wkT": bf(wk[g * 512:(g + 1) * 512, :].T),
            "wvT": bf(wv[g * 512:(g + 1) * 512, :].T),
            "woT": bf(wo[:, g * 1024:(g + 1) * 1024].T),
            "cosq": cos_q, "sinq": sin_q, "cosk": cos_k, "sink": sin_k,
            "mask": mask,
        })
    return in_maps


_NC_CACHE = {}


def kernel(x, cos, sin, wq, wk, wv, wo, q_norm_w, k_norm_w, _results=None,
           **run_kwargs):
    x = np.asarray(x, np.float32)
    in_maps = make_in_maps(np.asarray(x, np.float32), np.asarray(cos, np.float32),
                           np.asarray(sin, np.float32), np.asarray(wq, np.float32),
                           np.asarray(wk, np.float32), np.asarray(wv, np.float32),
                           np.asarray(wo, np.float32),
                           np.asarray(q_norm_w, np.float32),
                           np.asarray(k_norm_w, np.float32))
    if "nc" not in _NC_CACHE:
        _NC_CACHE["nc"] = build_program()
    nc = _NC_CACHE["nc"]
    res = run_bass_kernel_spmd(nc, in_maps, core_ids=list(range(8)),
                               **run_kwargs)
    if _results is not None:
        _results.append(res)
    B = x.shape[0]
    out = np.zeros((B, S, HSD), np.float32)
    for b in range(B):
        out[b] = res.results[2 * b]["out"] + res.results[2 * b + 1]["out"]
    return out


# revision 30
# speedup vs baseline: 1.0734x; 1.0734x over previous
"""Trainium2 Bass kernel for nn_Attention_21285857919576.

Strategy: 8 cores = 4 batches x 2 head-groups (tensor parallel over heads).
Each core computes, for its (batch b, head-group g):
  - Q/K/V projections (s-major) on TensorE in bf16
  - per-head RMSNorm (rsqrt via ACT exp(-0.5*ln(ms))) + RoPE on DVE/ACT/GpSimd
  - DMA-transpose roped Q,K into d-major slabs (QT/KT)
  - causal attention in transposed orientation:
      scoresT[k,q] = KT_tile.T @ QT  -> exp (no max subtraction: RMS+RoPE
      bounds |scores| <= sqrt(D)) -> p (bf16)
      outT[d,q]  += V_tile.T @ p     (V is s-major from the projection)
      den[1,q]   += ones.T @ p
      attnoutT = outT * (1/den broadcast via K=1 matmul)
  - partial output = attnoutT.T @ woT (this group's wo columns)
Host sums the two partial outputs per batch. No collectives; the 8 cores are
fully independent and perfectly load balanced.

All large matmuls run in bf16 (fp32 PSUM accumulate). Host-side
preprocessing folds q/k norm weights, the rotate-half sign, and the
1/sqrt(D) score scale into the cos/sin tables, and pre-transposes
x and the weights so every DMA is layout-natural.
"""
import sys
import numpy as np
import ml_dtypes

for _p in ("/opt/trn_rl_repo", "/opt/pypackages"):
    if _p not in sys.path:
        sys.path.append(_p)

import concourse.bass as bass
from concourse import bacc, mybir, tile
from concourse.bass_utils import run_bass_kernel_spmd


def _install_ntff_hook_shim():
    """The staged antenv package lacks axon_hooks; provide it so
    run_bass_kernel_spmd(trace=True) can drive NTFF profiling via the
    injected libaxon .so (same mechanism trn_boot would register)."""
    import types
    if "antenv.axon_hooks" in sys.modules:
        return
    mod = types.ModuleType("antenv.axon_hooks")
    _state = {"hook": None}
    mod.set_axon_ntff_profile_hook = lambda h: _state.__setitem__("hook", h)
    mod.get_axon_ntff_profile_hook = lambda: _state["hook"]
    sys.modules["antenv.axon_hooks"] = mod
    try:
        import antenv
        antenv.axon_hooks = mod
    except ImportError:
        pass
    try:
        from trn_agent_boot.trn_boot import _ntff_profile_via_ctypes
        import os
        so = "/opt/axon/libaxon_pjrt.so"
        if os.path.exists(so):
            mod.set_axon_ntff_profile_hook(_ntff_profile_via_ctypes(so))
    except Exception:
        pass


_install_ntff_hook_shim()


def _install_act_table_patch():
    """Force Exp/Ln/Copy/Square onto the single natural_log_exp_and_others
    ACT table set: blank every other set containing exp/ln so the
    table-load chooser can't alternate between sets (each switch costs
    ~2.7us and we interleave Ln (rms) with Exp (softmax))."""
    import concourse.hw_specs as hw_specs
    import concourse.bacc as bacc_mod
    if getattr(hw_specs, "_act_patch", False):
        return
    orig = hw_specs.get_activation_tables

    def patched(module_arch):
        tables = orig(module_arch)
        keep = "natural_log_exp_and_others"
        if keep in tables:
            for name, fns in tables.items():
                if name != keep and any(f.name in ("Exp", "Ln") for f in fns):
                    tables[name] = set()
        return tables

    hw_specs.get_activation_tables = patched
    if getattr(bacc_mod, "get_activation_tables", None) is orig:
        bacc_mod.get_activation_tables = patched
    hw_specs._act_patch = True


_install_act_table_patch()

BF = ml_dtypes.bfloat16
F32 = mybir.dt.float32
BF16 = mybir.dt.bfloat16
ALU = mybir.AluOpType
AF = mybir.ActivationFunctionType

S, HSD, D = 2048, 2048, 128
NQ, NKV = 8, 4          # per-core q heads / kv heads
NT = S // 128           # 16 s-tiles
EPS = 1e-6
MASKVAL = -1.0e9


def _emit(tc, aps):
    nc = tc.nc
    xT = aps["xT"]
    wqT = aps["wqT"]
    wkT = aps["wkT"]
    wvT = aps["wvT"]
    woT = aps["woT"]
    cosq = aps["cosq"]
    sinq = aps["sinq"]
    cosk = aps["cosk"]
    sink = aps["sink"]
    maskd = aps["mask"]
    outd = aps["out"]

    xT3 = xT.rearrange("(ho hi) s -> hi ho s", hi=128)       # [128, 16, 2048]
    wqT3 = wqT.rearrange("(ho hi) f -> hi ho f", hi=128)     # [128, 16, 1024]
    wkT3 = wkT.rearrange("(ho hi) f -> hi ho f", hi=128)     # [128, 16, 512]
    wvT3 = wvT.rearrange("(ho hi) f -> hi ho f", hi=128)
    woT3 = woT.rearrange("(fo fi) o -> fi fo o", fi=128)     # [128, 8, 2048]

    from contextlib import ExitStack
    with ExitStack() as ctx:
        singles = ctx.enter_context(tc.tile_pool(name="singles", bufs=1))
        wsl = ctx.enter_context(tc.tile_pool(name="wsl", bufs=2))
        xtp = ctx.enter_context(tc.tile_pool(name="xtp", bufs=2))
        trig = ctx.enter_context(tc.tile_pool(name="trig", bufs=4))
        qfp = ctx.enter_context(tc.tile_pool(name="qfp", bufs=3))
        t12 = ctx.enter_context(tc.tile_pool(name="t12", bufs=4))
        sqp = ctx.enter_context(tc.tile_pool(name="sqp", bufs=2))
        tiny = ctx.enter_context(tc.tile_pool(name="tiny", bufs=6))
        qsbp = ctx.enter_context(tc.tile_pool(name="qsbp", bufs=2))
        ksbp = ctx.enter_context(tc.tile_pool(name="ksbp", bufs=2))
        pp = ctx.enter_context(tc.tile_pool(name="pp", bufs=6))
        denp = ctx.enter_context(tc.tile_pool(name="denp", bufs=2))
        outp = ctx.enter_context(tc.tile_pool(name="outp", bufs=3))
        rdenp = ctx.enter_context(tc.tile_pool(name="rdenp", bufs=2))
        diagp = ctx.enter_context(tc.tile_pool(name="diagp", bufs=4))
        psA = ctx.enter_context(tc.tile_pool(name="psA", bufs=2, space="PSUM"))
        psSC = ctx.enter_context(tc.tile_pool(name="psSC", bufs=3, space="PSUM"))
        psO = ctx.enter_context(tc.tile_pool(name="psO", bufs=2, space="PSUM"))
        psD = ctx.enter_context(tc.tile_pool(name="psD", bufs=1, space="PSUM"))

        QT = singles.tile([128, NQ, S], BF16)    # [d, h, s]
        KT = singles.tile([128, NKV, S], BF16)   # [d, kv, s]
        Vs = singles.tile([128, NT, NKV * D], BF16)  # [s_i, s_o, f]
        AOT = singles.tile([128, NQ, S], BF16)   # [d, h, s]
        mask_sb = singles.tile([128, 128], F32)
        ones_col = singles.tile([128, 1], BF16)
        ones_row = singles.tile([1, 128], F32)

        eps_sb = singles.tile([128, 1], F32)
        ident = singles.tile([128, 128], F32)
        nc.sync.dma_start(out=mask_sb[:], in_=maskd)
        nc.vector.memset(ones_col[:], 1.0)
        nc.vector.memset(ones_row[:], 1.0)
        nc.vector.memset(eps_sb[:], EPS)
        from concourse.masks import make_identity
        make_identity(nc, ident[:])

        def process_qk(ps, cos_t, sin_t, out_ap):
            """RoPE 4 heads from psum tile ps [128,4,128] (unnormalized),
            write bf16 to out_ap [128, 512]; return rms_inv [128, 4] tile
            (applied later via the diag transpose matmul)."""
            qf = qfp.tile([128, 4, 128], F32, tag="qf")
            nc.scalar.copy(qf[:], ps[:])          # ACT: psum -> sbuf fp32
            sq = sqp.tile([128, 4, 128], F32, tag="sq")
            ss = tiny.tile([128, 4], F32, tag="ss")
            for hb in range(4):
                nc.vector.scalar_tensor_tensor(
                    out=sq[:, hb], in0=qf[:, hb], scalar=1.0, in1=qf[:, hb],
                    op0=ALU.mult, op1=ALU.mult, accum_out=ss[:, hb:hb + 1])
            lnt = tiny.tile([128, 4], F32, tag="lnt")
            nc.scalar.activation(lnt[:], ss[:], AF.Ln, scale=1.0 / D,
                                 bias=eps_sb[:])
            rmsi = tiny.tile([128, 4], F32, tag="rmsi")
            nc.scalar.activation(rmsi[:], lnt[:], AF.Exp, scale=-0.5)
            cosb = cos_t[:].unsqueeze(1).to_broadcast([128, 4, 128])
            sinb = sin_t[:].unsqueeze(1).to_broadcast([128, 4, 128])
            t1 = t12.tile([128, 4, 128], F32, tag="t1")
            t2 = t12.tile([128, 4, 128], F32, tag="t2")
            nc.vector.tensor_tensor(out=t1[:], in0=qf[:], in1=cosb, op=ALU.mult)
            qfb = qf[:]
            rot = bass.AP(tensor=qfb.tensor, offset=qfb.offset + 64,
                          ap=[qfb.ap[0], [128, 4], [-64, 2], [1, 64]])
            t2v = t2[:].rearrange("p h (j d) -> p h j d", j=2)
            sinv = sinb.rearrange("p h (j d) -> p h j d", j=2)
            nc.vector.tensor_tensor(out=t2v, in0=rot, in1=sinv, op=ALU.mult)
            nc.gpsimd.tensor_tensor(out=out_ap, in0=t1[:], in1=t2[:], op=ALU.add)
            return rmsi

        def transpose4(src_ap, rmsi, dst_ap):
            """Transpose+normalize 4 head-tiles: src_ap [128 s, 512 f] bf16
            (roped, unnormalized), rmsi [128 s, 4], into dst_ap
            [128 d, 4, 128 s] (bf16 slab slice) via PE diag matmuls."""
            pst = psSC.tile([128, 4, 128], F32, tag="sc")
            diag4 = diagp.tile([128, 4, 128], BF16, tag="diag")
            identb = ident[:].unsqueeze(1).to_broadcast([128, 4, 128])
            rmsib = rmsi[:].unsqueeze(2).to_broadcast([128, 4, 128])
            nc.vector.tensor_tensor(out=diag4[:], in0=identb, in1=rmsib,
                                    op=ALU.mult)
            for hb in range(4):
                nc.tensor.matmul(pst[:, hb],
                                 lhsT=src_ap[:, hb * 128:(hb + 1) * 128],
                                 rhs=diag4[:, hb], start=True, stop=True)
            nc.scalar.copy(dst_ap, pst[:])

        # ---- sweep 1: Q projection + rms/rope + transpose into QT ----
        wqA = wsl.tile([128, 16, 512], BF16, tag="w")
        wqB = wsl.tile([128, 16, 512], BF16, tag="w")
        nc.sync.dma_start(out=wqA[:], in_=wqT3[:, :, 0:512])
        nc.sync.dma_start(out=wqB[:], in_=wqT3[:, :, 512:1024])
        for st in range(NT):
            sl = slice(st * 128, (st + 1) * 128)
            xt = xtp.tile([128, 16, 128], BF16, tag="xt")
            nc.sync.dma_start(out=xt[:], in_=xT3[:, :, sl])
            ps0 = psA.tile([128, 4, 128], F32, tag="psA")
            ps1 = psA.tile([128, 4, 128], F32, tag="psA")
            for hs in range(16):
                nc.tensor.matmul(ps0[:], lhsT=xt[:, hs], rhs=wqA[:, hs],
                                 start=(hs == 0), stop=(hs == 15))
                nc.tensor.matmul(ps1[:], lhsT=xt[:, hs], rhs=wqB[:, hs],
                                 start=(hs == 0), stop=(hs == 15))
            ct = trig.tile([128, 128], F32, tag="trig")
            stt = trig.tile([128, 128], F32, tag="trig")
            nc.sync.dma_start(out=ct[:], in_=cosq[sl, :])
            nc.sync.dma_start(out=stt[:], in_=sinq[sl, :])
            qsb = qsbp.tile([128, 1024], BF16, tag="qsb")
            rmsi0 = process_qk(ps0, ct, stt, qsb[:, 0:512])
            rmsi1 = process_qk(ps1, ct, stt, qsb[:, 512:1024])
            transpose4(qsb[:, 0:512], rmsi0, QT[:, 0:4, sl])
            transpose4(qsb[:, 512:1024], rmsi1, QT[:, 4:8, sl])

        # ---- sweep 2: K/V projections; K rms/rope + transpose; V copy ----
        wkS = wsl.tile([128, 16, 512], BF16, tag="w")
        wvS = wsl.tile([128, 16, 512], BF16, tag="w")
        nc.sync.dma_start(out=wkS[:], in_=wkT3[:])
        nc.sync.dma_start(out=wvS[:], in_=wvT3[:])
        for st in range(NT):
            sl = slice(st * 128, (st + 1) * 128)
            xt = xtp.tile([128, 16, 128], BF16, tag="xt")
            nc.sync.dma_start(out=xt[:], in_=xT3[:, :, sl])
            psk = psA.tile([128, 4, 128], F32, tag="psA")
            psv = psA.tile([128, 4, 128], F32, tag="psA")
            for hs in range(16):
                nc.tensor.matmul(psk[:], lhsT=xt[:, hs], rhs=wkS[:, hs],
                                 start=(hs == 0), stop=(hs == 15))
                nc.tensor.matmul(psv[:], lhsT=xt[:, hs], rhs=wvS[:, hs],
                                 start=(hs == 0), stop=(hs == 15))
            ct = trig.tile([128, 128], F32, tag="trig")
            stt = trig.tile([128, 128], F32, tag="trig")
            nc.sync.dma_start(out=ct[:], in_=cosk[sl, :])
            nc.sync.dma_start(out=stt[:], in_=sink[sl, :])
            ksb = ksbp.tile([128, 512], BF16, tag="ksb")
            rmsik = process_qk(psk, ct, stt, ksb[:, 0:512])
            nc.scalar.copy(Vs[:, st, :], psv[:])
            transpose4(ksb[:, 0:512], rmsik, KT[:, 0:4, sl])

        # ---- attention (transposed orientation) + wo, interleaved by qc ----
        for qc in range(4):
            qbase = qc * 512
            for h in range(NQ):
                kv = h // 2
                nkt = 4 * qc + 4
                pso = psO.tile([128, 512], F32, tag="psO")
                psden = psD.tile([1, 512], F32, tag="psD")
                for kt in range(nkt):
                    j = kt - 4 * qc
                    lo = 128 * j if j >= 0 else 0
                    psc = psSC.tile([128, 512], F32, tag="sc")
                    nc.tensor.matmul(
                        psc[:, lo:512],
                        lhsT=KT[:, kv, kt * 128:(kt + 1) * 128],
                        rhs=QT[:, h, qbase + lo:qbase + 512],
                        start=True, stop=True)
                    if j >= 0:
                        nc.vector.tensor_tensor(
                            out=psc[:, lo:lo + 128], in0=psc[:, lo:lo + 128],
                            in1=mask_sb[:], op=ALU.add)
                    p = pp.tile([128, 512], BF16, tag="p")
                    nc.scalar.activation(p[:, lo:512], psc[:, lo:512], AF.Exp)
                    nc.tensor.matmul(
                        pso[:, lo:512],
                        lhsT=Vs[:, kt, kv * 128:(kv + 1) * 128],
                        rhs=p[:, lo:512],
                        start=(kt == 0), stop=(kt == nkt - 1))
                    nc.tensor.matmul(
                        psden[:, lo:512], lhsT=ones_col[:], rhs=p[:, lo:512],
                        start=(kt == 0), stop=(kt == nkt - 1))
                den_sb = denp.tile([1, 512], F32, tag="den")
                nc.vector.reciprocal_approx_fast(out=den_sb[:], in_=psden[:])
                rden = rdenp.tile([128, 512], F32, tag="rden")
                nc.gpsimd.partition_broadcast(rden[:], den_sb[:])
                nc.vector.tensor_tensor(
                    out=AOT[:, h, qbase:qbase + 512], in0=pso[:], in1=rden[:],
                    op=ALU.mult)

            # wo for the 4 s-tiles covered by this qc block
            if qc == 0:
                woA = wsl.tile([128, 4, 2048], BF16, tag="w")
                woB = wsl.tile([128, 4, 2048], BF16, tag="w")
                nc.sync.dma_start(out=woA[:], in_=woT3[:, 0:4, :])
                nc.sync.dma_start(out=woB[:], in_=woT3[:, 4:8, :])
            for st in range(4 * qc, 4 * qc + 4):
                ssl = slice(st * 128, (st + 1) * 128)
                for oc in range(4):
                    osl = slice(oc * 512, (oc + 1) * 512)
                    po = psO.tile([128, 512], F32, tag="psO")
                    for fc in range(NQ):
                        w = woA if fc < 4 else woB
                        nc.tensor.matmul(po[:], lhsT=AOT[:, fc, ssl],
                                         rhs=w[:, fc % 4, osl],
                                         start=(fc == 0), stop=(fc == 7))
                    osb = outp.tile([128, 512], F32, tag="osb")
                    nc.vector.tensor_copy(out=osb[:], in_=po[:])
                    nc.sync.dma_start(out=outd[ssl, osl], in_=osb[:])


def build_program():
    nc = bacc.Bacc("TRN2", target_bir_lowering=False, debug=False,
                   num_devices=8)
    shapes = {
        "xT": ([HSD, S], BF16), "wqT": ([HSD, NQ * D], BF16),
        "wkT": ([HSD, NKV * D], BF16), "wvT": ([HSD, NKV * D], BF16),
        "woT": ([NQ * D, HSD], BF16),
        "cosq": ([S, D], F32), "sinq": ([S, D], F32),
        "cosk": ([S, D], F32), "sink": ([S, D], F32),
        "mask": ([128, 128], F32),
    }
    aps = {n: nc.dram_tensor(n, sh, dt, kind="ExternalInput").ap()
           for n, (sh, dt) in shapes.items()}
    aps["out"] = nc.dram_tensor("out", [S, HSD], F32,
                                kind="ExternalOutput").ap()

    with tile.TileContext(nc) as tc:
        _emit(tc, aps)
    nc.compile()
    return nc


def make_in_maps(x, cos, sin, wq, wk, wv, wo, q_norm_w, k_norm_w):
    """Host-side preprocessing + sharding into 8 per-core input maps."""
    sign = np.where(np.arange(D) < 64, -1.0, 1.0).astype(np.float32)
    wrot_q = q_norm_w[(np.arange(D) + 64) % D]
    wrot_k = k_norm_w[(np.arange(D) + 64) % D]
    rsd = 1.0 / np.sqrt(np.float32(D))
    cos_q = (cos * q_norm_w[None, :] * rsd).astype(np.float32)
    sin_q = (sin * sign[None, :] * wrot_q[None, :] * rsd).astype(np.float32)
    cos_k = (cos * k_norm_w[None, :]).astype(np.float32)
    sin_k = (sin * sign[None, :] * wrot_k[None, :]).astype(np.float32)
    ii = np.arange(128)
    mask = np.where(ii[None, :] >= ii[:, None], 0.0, MASKVAL).astype(np.float32)

    def bf(a):
        return np.ascontiguousarray(a).astype(BF)

    in_maps = []
    for ci in range(8):
        b, g = ci // 2, ci % 2
        in_maps.append({
            "xT": bf(x[b].T),
            "wqT": bf(wq[g * 1024:(g + 1) * 1024, :].T),
            "wkT": bf(wk[g * 512:(g + 1) * 512, :].T),
            "wvT": bf(wv[g * 512:(g + 1) * 512, :].T),
            "woT": bf(wo[:, g * 1024:(g + 1) * 1024].T),
            "cosq": cos_q, "sinq": sin_q, "cosk": cos_k, "sink": sin_k,
            "mask": mask,
        })
    return in_maps


_NC_CACHE = {}


def kernel(x, cos, sin, wq, wk, wv, wo, q_norm_w, k_norm_w, _results=None,
           **run_kwargs):
    x = np.asarray(x, np.float32)
    in_maps = make_in_maps(np.asarray(x, np.float32), np.asarray(cos, np.float32),
                           np.asarray(sin, np.float32), np.asarray(wq, np.float32),
                           np.asarray(wk, np.float32), np.asarray(wv, np.float32),
                           np.asarray(wo, np.float32),
                           np.asarray(q_norm_w, np.float32),
                           np.asarray(k_norm_w, np.float32))
    if "nc" not in _NC_CACHE:
        _NC_CACHE["nc"] = build_program()
    nc = _NC_CACHE["nc"]
    res = run_bass_kernel_spmd(nc, in_maps, core_ids=list(range(8)),
                               **run_kwargs)
    if _results is not None:
        _results.append(res)
    B = x.shape[0]
    out = np.zeros((B, S, HSD), np.float32)
    for b in range(B):
        out[b] = res.results[2 * b]["out"] + res.results[2 * b + 1]["out"]
    return out


# revision 37
# speedup vs baseline: 1.1756x; 1.0951x over previous
"""Trainium2 Bass kernel for nn_Attention_21285857919576.

Strategy: 8 cores = 4 batches x 2 head-groups (tensor parallel over heads).
Each core computes, for its (batch b, head-group g):
  - Q/K/V projections (s-major) on TensorE in bf16
  - per-head RMSNorm (rsqrt via ACT exp(-0.5*ln(ms))) + RoPE on DVE/ACT/GpSimd
  - DMA-transpose roped Q,K into d-major slabs (QT/KT)
  - causal attention in transposed orientation:
      scoresT[k,q] = KT_tile.T @ QT  -> exp (no max subtraction: RMS+RoPE
      bounds |scores| <= sqrt(D)) -> p (bf16)
      outT[d,q]  += V_tile.T @ p     (V is s-major from the projection)
      den[1,q]   += ones.T @ p
      attnoutT = outT * (1/den broadcast via K=1 matmul)
  - partial output = attnoutT.T @ woT (this group's wo columns)
Host sums the two partial outputs per batch. No collectives; the 8 cores are
fully independent and perfectly load balanced.

All large matmuls run in bf16 (fp32 PSUM accumulate). Host-side
preprocessing folds q/k norm weights, the rotate-half sign, and the
1/sqrt(D) score scale into the cos/sin tables, and pre-transposes
x and the weights so every DMA is layout-natural.
"""
import sys
import numpy as np
import ml_dtypes

for _p in ("/opt/trn_rl_repo", "/opt/pypackages"):
    if _p not in sys.path:
        sys.path.append(_p)

import concourse.bass as bass
from concourse import bacc, mybir, tile
from concourse.bass_utils import run_bass_kernel_spmd


def _install_ntff_hook_shim():
    """The staged antenv package lacks axon_hooks; provide it so
    run_bass_kernel_spmd(trace=True) can drive NTFF profiling via the
    injected libaxon .so (same mechanism trn_boot would register)."""
    import types
    if "antenv.axon_hooks" in sys.modules:
        return
    mod = types.ModuleType("antenv.axon_hooks")
    _state = {"hook": None}
    mod.set_axon_ntff_profile_hook = lambda h: _state.__setitem__("hook", h)
    mod.get_axon_ntff_profile_hook = lambda: _state["hook"]
    sys.modules["antenv.axon_hooks"] = mod
    try:
        import antenv
        antenv.axon_hooks = mod
    except ImportError:
        pass
    try:
        from trn_agent_boot.trn_boot import _ntff_profile_via_ctypes
        import os
        so = "/opt/axon/libaxon_pjrt.so"
        if os.path.exists(so):
            mod.set_axon_ntff_profile_hook(_ntff_profile_via_ctypes(so))
    except Exception:
        pass


_install_ntff_hook_shim()


def _install_act_table_patch():
    """Force Exp/Ln/Copy/Square onto the single natural_log_exp_and_others
    ACT table set: blank every other set containing exp/ln so the
    table-load chooser can't alternate between sets (each switch costs
    ~2.7us and we interleave Ln (rms) with Exp (softmax))."""
    import concourse.hw_specs as hw_specs
    import concourse.bacc as bacc_mod
    if getattr(hw_specs, "_act_patch", False):
        return
    orig = hw_specs.get_activation_tables

    def patched(module_arch):
        tables = orig(module_arch)
        keep = "natural_log_exp_and_others"
        if keep in tables:
            for name, fns in tables.items():
                if name != keep and any(f.name in ("Exp", "Ln") for f in fns):
                    tables[name] = set()
        return tables

    hw_specs.get_activation_tables = patched
    if getattr(bacc_mod, "get_activation_tables", None) is orig:
        bacc_mod.get_activation_tables = patched
    hw_specs._act_patch = True


_install_act_table_patch()

BF = ml_dtypes.bfloat16
F32 = mybir.dt.float32
BF16 = mybir.dt.bfloat16
ALU = mybir.AluOpType
AF = mybir.ActivationFunctionType

S, HSD, D = 2048, 2048, 128
NQ, NKV = 8, 4          # per-core q heads / kv heads
NT = S // 128           # 16 s-tiles
EPS = 1e-6
MASKVAL = -1.0e9


def _emit(tc, aps):
    nc = tc.nc
    xT = aps["xT"]
    wqT = aps["wqT"]
    wkT = aps["wkT"]
    wvT = aps["wvT"]
    woT = aps["woT"]
    cosq = aps["cosq"]
    sinq = aps["sinq"]
    cosk = aps["cosk"]
    sink = aps["sink"]
    maskd = aps["mask"]
    outd = aps["out"]

    xT3 = xT.rearrange("(ho hi) s -> hi ho s", hi=128)       # [128, 16, 2048]
    wqT3 = wqT.rearrange("(ho hi) f -> hi ho f", hi=128)     # [128, 16, 1024]
    wkT3 = wkT.rearrange("(ho hi) f -> hi ho f", hi=128)     # [128, 16, 512]
    wvT3 = wvT.rearrange("(ho hi) f -> hi ho f", hi=128)
    woT3 = woT.rearrange("(fo fi) o -> fi fo o", fi=128)     # [128, 8, 2048]

    from contextlib import ExitStack
    with ExitStack() as ctx:
        singles = ctx.enter_context(tc.tile_pool(name="singles", bufs=1))
        wsl = ctx.enter_context(tc.tile_pool(name="wsl", bufs=2))
        xtp = ctx.enter_context(tc.tile_pool(name="xtp", bufs=2))
        trig = ctx.enter_context(tc.tile_pool(name="trig", bufs=4))
        qfp = ctx.enter_context(tc.tile_pool(name="qfp", bufs=3))
        t12 = ctx.enter_context(tc.tile_pool(name="t12", bufs=4))
        sqp = ctx.enter_context(tc.tile_pool(name="sqp", bufs=2))
        tiny = ctx.enter_context(tc.tile_pool(name="tiny", bufs=8))
        qsbp = ctx.enter_context(tc.tile_pool(name="qsbp", bufs=3))
        ksbp = ctx.enter_context(tc.tile_pool(name="ksbp", bufs=3))
        pp = ctx.enter_context(tc.tile_pool(name="pp", bufs=6))
        denp = ctx.enter_context(tc.tile_pool(name="denp", bufs=2))
        outp = ctx.enter_context(tc.tile_pool(name="outp", bufs=3))
        rdenp = ctx.enter_context(tc.tile_pool(name="rdenp", bufs=2))
        diagp = ctx.enter_context(tc.tile_pool(name="diagp", bufs=4))
        psA = ctx.enter_context(tc.tile_pool(name="psA", bufs=2, space="PSUM"))
        psSC = ctx.enter_context(tc.tile_pool(name="psSC", bufs=3, space="PSUM"))
        psO = ctx.enter_context(tc.tile_pool(name="psO", bufs=2, space="PSUM"))
        psD = ctx.enter_context(tc.tile_pool(name="psD", bufs=1, space="PSUM"))

        QT = singles.tile([128, NQ, S], BF16)    # [d, h, s]
        KT = singles.tile([128, NKV, S], BF16)   # [d, kv, s]
        Vs = singles.tile([128, NT, NKV * D], BF16)  # [s_i, s_o, f]
        AOT = singles.tile([128, NQ, S], BF16)   # [d, h, s]
        mask_sb = singles.tile([128, 128], F32)
        ones_col = singles.tile([128, 1], BF16)
        ones_row = singles.tile([1, 128], F32)

        eps_sb = singles.tile([128, 1], F32)
        ident = singles.tile([128, 128], F32)
        nc.sync.dma_start(out=mask_sb[:], in_=maskd)
        nc.vector.memset(ones_col[:], 1.0)
        nc.vector.memset(ones_row[:], 1.0)
        nc.vector.memset(eps_sb[:], EPS)
        from concourse.masks import make_identity
        make_identity(nc, ident[:])

        def process_qk(ps, cos_t, sin_t, out_ap):
            """RoPE 4 heads from psum tile ps [128,4,128] (unnormalized),
            write bf16 to out_ap [128, 512]; return rms_inv [128, 4] tile
            (applied later via the diag transpose matmul)."""
            qf = qfp.tile([128, 4, 128], F32, tag="qf")
            nc.scalar.copy(qf[:], ps[:])          # ACT: psum -> sbuf fp32
            sq = sqp.tile([128, 4, 128], F32, tag="sq")
            ss = tiny.tile([128, 4], F32, tag="ss")
            for hb in range(4):
                nc.vector.scalar_tensor_tensor(
                    out=sq[:, hb], in0=qf[:, hb], scalar=1.0, in1=qf[:, hb],
                    op0=ALU.mult, op1=ALU.mult, accum_out=ss[:, hb:hb + 1])
            lnt = tiny.tile([128, 4], F32, tag="lnt")
            nc.scalar.activation(lnt[:], ss[:], AF.Ln, scale=1.0 / D,
                                 bias=eps_sb[:])
            rmsi = tiny.tile([128, 4], F32, tag="rmsi")
            nc.scalar.activation(rmsi[:], lnt[:], AF.Exp, scale=-0.5)
            cosb = cos_t[:].unsqueeze(1).to_broadcast([128, 4, 128])
            sinb = sin_t[:].unsqueeze(1).to_broadcast([128, 4, 128])
            t1 = t12.tile([128, 4, 128], F32, tag="t1")
            t2 = t12.tile([128, 4, 128], F32, tag="t2")
            nc.vector.tensor_tensor(out=t1[:], in0=qf[:], in1=cosb, op=ALU.mult)
            qfb = qf[:]
            rot = bass.AP(tensor=qfb.tensor, offset=qfb.offset + 64,
                          ap=[qfb.ap[0], [128, 4], [-64, 2], [1, 64]])
            t2v = t2[:].rearrange("p h (j d) -> p h j d", j=2)
            sinv = sinb.rearrange("p h (j d) -> p h j d", j=2)
            nc.vector.tensor_tensor(out=t2v, in0=rot, in1=sinv, op=ALU.mult)
            nc.gpsimd.tensor_tensor(out=out_ap, in0=t1[:], in1=t2[:], op=ALU.add)
            return rmsi

        def transpose4(src_ap, rmsi, dst_ap):
            """Transpose+normalize 4 head-tiles: src_ap [128 s, 512 f] bf16
            (roped, unnormalized), rmsi [128 s, 4], into dst_ap
            [128 d, 4, 128 s] (bf16 slab slice) via PE diag matmuls."""
            pst = psSC.tile([128, 4, 128], F32, tag="sc")
            diag4 = diagp.tile([128, 4, 128], BF16, tag="diag")
            identb = ident[:].unsqueeze(1).to_broadcast([128, 4, 128])
            rmsib = rmsi[:].unsqueeze(2).to_broadcast([128, 4, 128])
            nc.vector.tensor_tensor(out=diag4[:], in0=identb, in1=rmsib,
                                    op=ALU.mult)
            for hb in range(4):
                nc.tensor.matmul(pst[:, hb],
                                 lhsT=src_ap[:, hb * 128:(hb + 1) * 128],
                                 rhs=diag4[:, hb], start=True, stop=True)
            nc.scalar.copy(dst_ap, pst[:])

        # ---- sweep 1: Q projection + rms/rope + transpose into QT ----
        pend_q = []
        wqA = wsl.tile([128, 16, 512], BF16, tag="w")
        wqB = wsl.tile([128, 16, 512], BF16, tag="w")
        nc.sync.dma_start(out=wqA[:], in_=wqT3[:, :, 0:512])
        nc.sync.dma_start(out=wqB[:], in_=wqT3[:, :, 512:1024])
        for st in range(NT):
            sl = slice(st * 128, (st + 1) * 128)
            xt = xtp.tile([128, 16, 128], BF16, tag="xt")
            nc.sync.dma_start(out=xt[:], in_=xT3[:, :, sl])
            ps0 = psA.tile([128, 4, 128], F32, tag="psA")
            ps1 = psA.tile([128, 4, 128], F32, tag="psA")
            for hs in range(16):
                nc.tensor.matmul(ps0[:], lhsT=xt[:, hs], rhs=wqA[:, hs],
                                 start=(hs == 0), stop=(hs == 15))
                nc.tensor.matmul(ps1[:], lhsT=xt[:, hs], rhs=wqB[:, hs],
                                 start=(hs == 0), stop=(hs == 15))
            ct = trig.tile([128, 128], F32, tag="trig")
            stt = trig.tile([128, 128], F32, tag="trig")
            nc.sync.dma_start(out=ct[:], in_=cosq[sl, :])
            nc.sync.dma_start(out=stt[:], in_=sinq[sl, :])
            qsb = qsbp.tile([128, 1024], BF16, tag="qsb")
            rmsi0 = process_qk(ps0, ct, stt, qsb[:, 0:512])
            rmsi1 = process_qk(ps1, ct, stt, qsb[:, 512:1024])
            pend_q.append((qsb, rmsi0, rmsi1, sl))
            if len(pend_q) > 1:
                pqsb, pr0, pr1, psl = pend_q.pop(0)
                transpose4(pqsb[:, 0:512], pr0, QT[:, 0:4, psl])
                transpose4(pqsb[:, 512:1024], pr1, QT[:, 4:8, psl])

        for pqsb, pr0, pr1, psl in pend_q:
            transpose4(pqsb[:, 0:512], pr0, QT[:, 0:4, psl])
            transpose4(pqsb[:, 512:1024], pr1, QT[:, 4:8, psl])
        pend_q = []

        # ---- sweep 2: K/V projections; K rms/rope + transpose; V copy ----
        pend_k = []
        wkS = wsl.tile([128, 16, 512], BF16, tag="w")
        wvS = wsl.tile([128, 16, 512], BF16, tag="w")
        nc.sync.dma_start(out=wkS[:], in_=wkT3[:])
        nc.sync.dma_start(out=wvS[:], in_=wvT3[:])
        for st in range(NT):
            sl = slice(st * 128, (st + 1) * 128)
            xt = xtp.tile([128, 16, 128], BF16, tag="xt")
            nc.sync.dma_start(out=xt[:], in_=xT3[:, :, sl])
            psk = psA.tile([128, 4, 128], F32, tag="psA")
            psv = psA.tile([128, 4, 128], F32, tag="psA")
            for hs in range(16):
                nc.tensor.matmul(psk[:], lhsT=xt[:, hs], rhs=wkS[:, hs],
                                 start=(hs == 0), stop=(hs == 15))
                nc.tensor.matmul(psv[:], lhsT=xt[:, hs], rhs=wvS[:, hs],
                                 start=(hs == 0), stop=(hs == 15))
            ct = trig.tile([128, 128], F32, tag="trig")
            stt = trig.tile([128, 128], F32, tag="trig")
            nc.sync.dma_start(out=ct[:], in_=cosk[sl, :])
            nc.sync.dma_start(out=stt[:], in_=sink[sl, :])
            ksb = ksbp.tile([128, 512], BF16, tag="ksb")
            rmsik = process_qk(psk, ct, stt, ksb[:, 0:512])
            nc.scalar.copy(Vs[:, st, :], psv[:])
            pend_k.append((ksb, rmsik, sl))
            if len(pend_k) > 1:
                pksb, prk, psl = pend_k.pop(0)
                transpose4(pksb[:, 0:512], prk, KT[:, 0:4, psl])

        for pksb, prk, psl in pend_k:
            transpose4(pksb[:, 0:512], prk, KT[:, 0:4, psl])
        pend_k = []

        # ---- attention (transposed orientation) + wo, interleaved by qc ----
        for qc in range(4):
            qbase = qc * 512
            for h in range(NQ):
                kv = h // 2
                nkt = 4 * qc + 4
                pso = psO.tile([128, 512], F32, tag="psO")
                psden = psD.tile([1, 512], F32, tag="psD")
                for kt in range(nkt):
                    j = kt - 4 * qc
                    lo = 128 * j if j >= 0 else 0
                    psc = psSC.tile([128, 512], F32, tag="sc")
                    nc.tensor.matmul(
                        psc[:, lo:512],
                        lhsT=KT[:, kv, kt * 128:(kt + 1) * 128],
                        rhs=QT[:, h, qbase + lo:qbase + 512],
                        start=True, stop=True)
                    if j >= 0:
                        nc.vector.tensor_tensor(
                            out=psc[:, lo:lo + 128], in0=psc[:, lo:lo + 128],
                            in1=mask_sb[:], op=ALU.add)
                    p = pp.tile([128, 512], BF16, tag="p")
                    nc.scalar.activation(p[:, lo:512], psc[:, lo:512], AF.Exp)
                    nc.tensor.matmul(
                        pso[:, lo:512],
                        lhsT=Vs[:, kt, kv * 128:(kv + 1) * 128],
                        rhs=p[:, lo:512],
                        start=(kt == 0), stop=(kt == nkt - 1))
                    nc.tensor.matmul(
                        psden[:, lo:512], lhsT=ones_col[:], rhs=p[:, lo:512],
                        start=(kt == 0), stop=(kt == nkt - 1))
                den_sb = denp.tile([1, 512], F32, tag="den")
                nc.vector.reciprocal_approx_fast(out=den_sb[:], in_=psden[:])
                rden = rdenp.tile([128, 512], F32, tag="rden")
                nc.gpsimd.partition_broadcast(rden[:], den_sb[:])
                nc.vector.tensor_tensor(
                    out=AOT[:, h, qbase:qbase + 512], in0=pso[:], in1=rden[:],
                    op=ALU.mult)

            # wo for the 4 s-tiles covered by this qc block
            if qc == 0:
                woA = wsl.tile([128, 4, 2048], BF16, tag="w")
                woB = wsl.tile([128, 4, 2048], BF16, tag="w")
                nc.sync.dma_start(out=woA[:], in_=woT3[:, 0:4, :])
                nc.sync.dma_start(out=woB[:], in_=woT3[:, 4:8, :])
            for st in range(4 * qc, 4 * qc + 4):
                ssl = slice(st * 128, (st + 1) * 128)
                for oc in range(4):
                    osl = slice(oc * 512, (oc + 1) * 512)
                    po = psO.tile([128, 512], F32, tag="psO")
                    for fc in range(NQ):
                        w = woA if fc < 4 else woB
                        nc.tensor.matmul(po[:], lhsT=AOT[:, fc, ssl],
                                         rhs=w[:, fc % 4, osl],
                                         start=(fc == 0), stop=(fc == 7))
                    osb = outp.tile([128, 512], F32, tag="osb")
                    nc.vector.tensor_copy(out=osb[:], in_=po[:])
                    nc.sync.dma_start(out=outd[ssl, osl], in_=osb[:])


def build_program():
    nc = bacc.Bacc("TRN2", target_bir_lowering=False, debug=False,
                   num_devices=8)
    shapes = {
        "xT": ([HSD, S], BF16), "wqT": ([HSD, NQ * D], BF16),
        "wkT": ([HSD, NKV * D], BF16), "wvT": ([HSD, NKV * D], BF16),
        "woT": ([NQ * D, HSD], BF16),
        "cosq": ([S, D], F32), "sinq": ([S, D], F32),
        "cosk": ([S, D], F32), "sink": ([S, D], F32),
        "mask": ([128, 128], F32),
    }
    aps = {n: nc.dram_tensor(n, sh, dt, kind="ExternalInput").ap()
           for n, (sh, dt) in shapes.items()}
    aps["out"] = nc.dram_tensor("out", [S, HSD], F32,
                                kind="ExternalOutput").ap()

    with tile.TileContext(nc) as tc:
        _emit(tc, aps)
    nc.compile()
    return nc


def make_in_maps(x, cos, sin, wq, wk, wv, wo, q_norm_w, k_norm_w):
    """Host-side preprocessing + sharding into 8 per-core input maps."""
    sign = np.where(np.arange(D) < 64, -1.0, 1.0).astype(np.float32)
    wrot_q = q_norm_w[(np.arange(D) + 64) % D]
    wrot_k = k_norm_w[(np.arange(D) + 64) % D]
    rsd = 1.0 / np.sqrt(np.float32(D))
    cos_q = (cos * q_norm_w[None, :] * rsd).astype(np.float32)
    sin_q = (sin * sign[None, :] * wrot_q[None, :] * rsd).astype(np.float32)
    cos_k = (cos * k_norm_w[None, :]).astype(np.float32)
    sin_k = (sin * sign[None, :] * wrot_k[None, :]).astype(np.float32)
    ii = np.arange(128)
    mask = np.where(ii[None, :] >= ii[:, None], 0.0, MASKVAL).astype(np.float32)

    def bf(a):
        return np.ascontiguousarray(a).astype(BF)

    in_maps = []
    for ci in range(8):
        b, g = ci // 2, ci % 2
        in_maps.append({
            "xT": bf(x[b].T),
            "wqT": bf(wq[g * 1024:(g + 1) * 1024, :].T),
            "wkT": bf(wk[g * 512:(g + 1) * 512, :].T),
            "wvT": bf(wv[g * 512:(g + 1) * 512, :].T),
            "woT": bf(wo[:, g * 1024:(g + 1) * 1024].T),
            "cosq": cos_q, "sinq": sin_q, "cosk": cos_k, "sink": sin_k,
            "mask": mask,
        })
    return in_maps


_NC_CACHE = {}


def kernel(x, cos, sin, wq, wk, wv, wo, q_norm_w, k_norm_w, _results=None,
           **run_kwargs):
    x = np.asarray(x, np.float32)
    in_maps = make_in_maps(np.asarray(x, np.float32), np.asarray(cos, np.float32),
                           np.asarray(sin, np.float32), np.asarray(wq, np.float32),
                           np.asarray(wk, np.float32), np.asarray(wv, np.float32),
                           np.asarray(wo, np.float32),
                           np.asarray(q_norm_w, np.float32),
                           np.asarray(k_norm_w, np.float32))
    if "nc" not in _NC_CACHE:
        _NC_CACHE["nc"] = build_program()
    nc = _NC_CACHE["nc"]
    res = run_bass_kernel_spmd(nc, in_maps, core_ids=list(range(8)),
                               **run_kwargs)
    if _results is not None:
        _results.append(res)
    B = x.shape[0]
    out = np.zeros((B, S, HSD), np.float32)
    for b in range(B):
        out[b] = res.results[2 * b]["out"] + res.results[2 * b + 1]["out"]
    return out
